# revision 43
# baseline (speedup 1.0000x reference)
"""Trainium2 Bass kernel for the Haar-mask MLP (histogram_binning).

Key algorithmic fact: every Haar interval edge is a multiple of 2^-10, so the
reference's masks -- and therefore the entire MLP output -- depend only on
u = floor(t * 1024) (1024 possible values, exact in fp32 since *1024 is a
power-of-two scale).  The whole network collapses to a 1024x3 lookup table,
computed once on host from the tiny weights.

Device design (default impl "pe", ~40us vs ~74us for the gpsimd gather):
the GpSimd gather runs at only ~25ns/index per q7 core (2048 indices/group
= ~52us, measured; see impl "ib1:c4" for the tuned gather variant), so the
LUT lookup is instead done on the TENSOR engine as a one-hot matmul:

  u = 8q + r,  q in [0,128), r in [0,8)
  - q = floor(128 t), u = floor(1024 t) exactly on DVE (int round-trip with
    a compare fixup works under any convert rounding mode); r = u - 8q;
    bit masks of r as uint8.
  - the host supplies t twice (natural and transposed): q is computed in
    the transposed frame, DMA'd partition-major to a DRAM row, and DMA'd
    back with a stride-0-partition broadcast AP -> q replicated on 128
    partitions, elements on the free axis (bins-on-partitions layout)
    with zero on-device transposes.
  - one-hot OH[k, e] = (q_e == k) via ONE tensor_scalar is_equal per
    generation with a per-partition iota scalar (DVE 2x bf16).
  - matmul per 128-element chunk: out[e, 3r+f] = OH_chunk.T @ Lr where
    Lr[k, 3r+f] = LUT[8k+r, f] (bf16, host input).  PSUM holds 16-chunk
    generations, double buffered.
  - a 3-stage DVE select tree on r's bits picks the element's 3 features;
    bf16 out DMA (well inside the 2e-2 rel-err budget).

Host does: LUT build (weights only), input layout permute, output unpermute.

Measured notes (HW): gpsimd indirect_copy/ap_gather ~25ns/index regardless
of dtype/chunking; indirect_copy dst limited to 1024 elems/partition;
ap_gather library load ~55us.  Framework floor (barriers, program loads,
DMA sem latency, epilogue) ~15us.

Impl strings kept for experiments: "pe" (default), "ic1/ib1/ap1[:cK][:vN]"
gather variants (fp32/bf16 indirect_copy, ap_gather).
"""

from contextlib import ExitStack

import numpy as np

import concourse.tile as tile
from concourse import bacc, library_config, mybir
from concourse.bass_utils import run_bass_kernel_spmd

N_CORES = 8
B, T, F = 16, 8192, 3
N = B * T                    # 131072 total elements
NPC = N // N_CORES           # 16384 per neuron core
P = 128                      # SBUF partitions
S = NPC // P                 # 128 slots per partition
NBINS = 1024
EPC = NPC // 8               # 2048 elements per q7 core

GATHER_IMPL = "pe"
RUN_KWARGS = {}              # test harness may set {"trace": True}
LAST_RESULTS = None
_CACHE = {}


def _build_lut(W1, b1, W2, b2, W3, b3):
    """MLP output for each of the 1024 half-interval bins, fp32 math."""
    u = np.arange(NBINS)
    acc = np.zeros((NBINS, W1.shape[1]), np.float32)
    for j in range(10):
        k = u >> (10 - j)                       # floor(t * 2^j) for t in bin u
        idx = (1 << j) - 1 + k                  # level-j block offset + k
        sign = np.where((u >> (9 - j)) & 1 == 0, np.float32(1), np.float32(-1))
        acc = acc + sign[:, None] * W1[idx]
    h = np.maximum(acc + b1, np.float32(0))
    h = np.maximum(h @ W2 + b2, np.float32(0))
    return (h @ W3 + b3).astype(np.float32)     # (1024, 3)


def _parse(impl):
    parts = impl.split(":")
    kind = parts[0]
    nchunk = 1
    preload = True
    nvalid = EPC
    for p in parts[1:]:
        if p.startswith("c"):
            nchunk = int(p[1:])
        elif p.startswith("v"):
            nvalid = int(p[1:])
        elif p == "nopre":
            preload = False
    return kind, nchunk, preload, nvalid


def _build_nc(impl):
    kind, nchunk, preload, nvalid = _parse(impl)
    use_ic = kind.startswith("i")
    gdt_np = np.dtype("bfloat16") if "b" in kind else np.float32

    nc = bacc.Bacc("TRN2", target_bir_lowering=False, debug=False,
                   enable_asserts=False, num_devices=N_CORES)
    f32 = mybir.dt.float32
    gdt = mybir.dt.bfloat16 if "b" in kind else f32
    idt = mybir.dt.uint16 if use_ic else mybir.dt.int16
    t_d = nc.dram_tensor("t", [P, S], f32, kind="ExternalInput")
    lut_d = nc.dram_tensor("lut", [P, NBINS], gdt, kind="ExternalInput")
    out_d = nc.dram_tensor("out", [F, 8, EPC], gdt, kind="ExternalOutput")

    with tile.TileContext(nc) as tc, ExitStack() as ctx:
        cpool = ctx.enter_context(tc.tile_pool(name="c", bufs=1))
        gpool = ctx.enter_context(tc.tile_pool(name="g", bufs=1))

        if preload and not use_ic:
            # overlap the one-time ap_gather ucode load with the prologue
            nc.gpsimd.load_library(library_config.ap_gather)

        # t alone on the sync queue (gates the DVE index chain), the whole
        # table as ONE dma on scalar (HWDGE issue cost dominates transfer
        # for these sizes -- one 256KB bf16 dma beats four quarter dmas)
        t_sb = cpool.tile([P, S], f32)
        nc.sync.dma_start(t_sb[:], t_d[:, :])
        t2_sb = cpool.tile([P, S], f32)
        nc.scalar.dma_start(t2_sb[:], t2_d[:, :])
        tab = cpool.tile([P, NBINS], gdt)
        nc.scalar.dma_start(tab[:], lut_d[:, :])

        spc = S // nchunk            # idx columns per chunk
        w = 16 * spc                 # gathered elements per group per chunk
        oeng = (nc.sync, nc.scalar, nc.sync)
        for k in range(nchunk):
            nv = nvalid if nchunk == 1 else w
            t_k = t_sb[:, k * spc:(k + 1) * spc]

            # exact floor(t*1024) in 4 DVE ops per chunk (separate tiles per
            # chunk so gather k only waits on its own chunk's index compute):
            # ii = int(t*1024) (any rounding mode), fb = float(ii),
            # adj = (fb*2^-10 > t) i.e. rounded-up, idx = fb-adj.
            # Result is always in [0, 1023] for t in [0,1), so no clamp.
            ii = cpool.tile([P, spc], mybir.dt.int32, tag=f"ii{k}")
            fb = cpool.tile([P, spc], f32, tag=f"fb{k}")
            adj = cpool.tile([P, spc], f32, tag=f"adj{k}")
            idx = cpool.tile([P, spc], idt, tag=f"idx{k}")
            nc.vector.tensor_scalar(ii[:], t_k, 1024.0, None,
                                    mybir.AluOpType.mult)
            nc.vector.tensor_copy(fb[:], ii[:])
            nc.vector.scalar_tensor_tensor(adj[:], fb[:], 2.0 ** -10, t_k,
                                           mybir.AluOpType.mult,
                                           mybir.AluOpType.is_gt)
            nc.vector.scalar_tensor_tensor(idx[:], fb[:], 1.0, adj[:],
                                           mybir.AluOpType.mult,
                                           mybir.AluOpType.subtract)

            g = gpool.tile([P, w], gdt, tag=f"g{k}")
            if use_ic:
                nc.gpsimd.indirect_copy(
                    g[:, :nv].rearrange("p (n d) -> p n d", d=1),
                    tab[:].rearrange("p (n d) -> p n d", d=1),
                    idx[:], i_know_ap_gather_is_preferred=True)
            else:
                nc.gpsimd.ap_gather(g[:, :nv], tab[:], idx[:],
                                    channels=P, num_elems=NBINS,
                                    d=1, num_idxs=nv)
            # only partitions 16c+f (f<3) hold useful data; one 8-partition
            # strided DMA per feature plane, each on its own engine queue
            for f in range(F):
                oeng[f].dma_start(out_d.ap()[f, :, k * w:k * w + nv],
                                  g[f:P:16, :nv])
    nc.compile()
    return nc


K_PE = 128                   # hi-bins (one-hot width / contraction dim)
R_PE = NBINS // K_PE         # 16 lo-values selected by the DVE tree
CHG = 32                     # matmul chunks (s-slots) per PSUM generation


def _build_pe():
    """Gather-free path: out = onehot(q) @ Lr via the tensor engine.

    u = floor(1024t) = 16q + r.  A (q+1)-scaled one-hot over K_PE=64 bins is
    built bins-on-partitions with affine_select (iota == q+1 keeps q+1, else
    0); Lr rows are pre-divided by (q+1) on host so the matmul yields exactly
    B[e, 3r+f] = LUT[16q_e + r, f].  A 4-stage DVE select tree then picks r_e.
    The bins-on-partitions layout comes from PE-transpose + DRAM round-trip
    with a stride-0 broadcast read.
    """
    nc = bacc.Bacc("TRN2", target_bir_lowering=False, debug=False,
                   enable_asserts=False, num_devices=N_CORES)
    f32 = mybir.dt.float32
    bf16 = mybir.dt.bfloat16
    t_d = nc.dram_tensor("t", [P, S], f32, kind="ExternalInput")
    t2_d = nc.dram_tensor("t2", [P, S], f32, kind="ExternalInput")
    lrp_d = nc.dram_tensor("lrp", [K_PE, 3 * R_PE], bf16, kind="ExternalInput")
    io_d = nc.dram_tensor("io64", [K_PE, 1], f32, kind="ExternalInput")
    out_d = nc.dram_tensor("out", [P, S, F], bf16, kind="ExternalOutput")
    scr_d = nc.dram_tensor("scr", [1, NPC], bf16, kind="Internal")

    with tile.TileContext(nc) as tc, ExitStack() as ctx:
        cpool = ctx.enter_context(tc.tile_pool(name="c", bufs=1))
        ppool = ctx.enter_context(tc.psum_pool(name="pp", bufs=2))

        t_sb = cpool.tile([P, S], f32)
        nc.sync.dma_start(t_sb[:], t_d[:, :])
        t2_sb = cpool.tile([P, S], f32)
        nc.scalar.dma_start(t2_sb[:], t2_d[:, :])
        lrp = cpool.tile([K_PE, 3 * R_PE], bf16)
        nc.scalar.dma_start(lrp[:], lrp_d[:, :])
        io64 = cpool.tile([K_PE, 1], f32)
        nc.scalar.dma_start(io64[:], io_d[:, :])

        TS, STT = nc.vector.tensor_scalar, nc.vector.scalar_tensor_tensor
        Op = mybir.AluOpType

        def guarded_floor(out, src, scale, tag):
            # out = floor(src*scale) exactly, any int-convert rounding mode
            iw = cpool.tile([P, S], mybir.dt.int32, tag=f"i{tag}")
            fw = cpool.tile([P, S], f32, tag=f"f{tag}")
            aw = cpool.tile([P, S], f32, tag=f"a{tag}")
            TS(iw[:], src, float(scale), None, Op.mult)
            nc.vector.tensor_copy(fw[:], iw[:])
            STT(aw[:], fw[:], 1.0 / scale, src, Op.mult, Op.is_gt)
            STT(out, fw[:], 1.0, aw[:], Op.mult, Op.subtract)

        u = cpool.tile([P, S], f32)
        q1 = cpool.tile([P, S], bf16)
        # computed in the TRANSPOSED frame (t2[s,p] = t[p,s]) so a plain
        # partition-major DMA to DRAM yields the s-major flat order the
        # broadcast wants -- no PE transpose / PSUM evac hop needed
        guarded_floor(q1[:], t2_sb[:], float(K_PE), "q")  # bf16 out, exact
        nc.sync.dma_start(scr_d.ap()[:, :], q1[:])

        guarded_floor(u[:], t_sb[:], 1024.0, "u")
        qn = cpool.tile([P, S], f32)
        guarded_floor(qn[:], t_sb[:], float(K_PE), "qn")
        r = cpool.tile([P, S], f32)
        STT(r[:], qn[:], -float(R_PE), u[:], Op.mult, Op.add)
        # bit masks of r for the select tree (uint8 0/1 -- CopyPredicated
        # requires an integer mask dtype); remainders via fmod, exact on
        # small integer-valued floats
        bits = []
        cur = r
        for v in (4.0, 2.0):
            b = cpool.tile([P, S], mybir.dt.uint8, tag=f"b{int(v)}")
            nxt = cpool.tile([P, S], f32, tag=f"r{int(v)}")
            TS(b[:], cur[:], v, None, Op.is_ge)
            STT(nxt[:], b[:], -v, cur[:], Op.mult, Op.add)
            bits.append(b)
            cur = nxt
        b0 = cpool.tile([P, S], mybir.dt.uint8, tag="b1")
        TS(b0[:], cur[:], 1.0, None, Op.is_ge)
        bits.append(b0)

        NG = S // CHG                             # generations
        W = P * CHG                               # elements per generation
        deng = (nc.sync, nc.scalar)
        for gi in range(NG):
            q1R = cpool.tile([K_PE, W], bf16, tag=f"q1R{gi}")
            deng[gi % 2].dma_start(
                q1R[:], scr_d.ap()[:, gi * W:(gi + 1) * W].to_broadcast([K_PE, W]))
            oh = cpool.tile([K_PE, W], bf16, tag=f"oh{gi}")
            TS(oh[:], q1R[:], io64[:], None, Op.is_equal)
            bp = ppool.tile([P, CHG, 64], f32, tag="B")
            for c in range(CHG):
                nc.tensor.matmul(bp[:, c, 0:3 * R_PE],
                                 oh[:, c * P:(c + 1) * P], lrp[:],
                                 start=True, stop=True)
            # 4-stage select tree over r bits: 48 -> 24 -> 12 -> 6 -> 3
            sl = gi * CHG
            src = bp
            wsel = 3 * R_PE
            for st, b in enumerate(bits):
                wsel //= 2
                dt_s = bf16
                dst = cpool.tile([P, CHG, wsel], dt_s, tag=f"s{st}_{gi}")
                m = b[:, sl:sl + CHG].unsqueeze(2).to_broadcast([P, CHG, wsel])
                nc.vector.select(dst[:], m, src[:, :, wsel:2 * wsel],
                                 src[:, :, 0:wsel])
                src = dst
            nc.scalar.dma_start(out_d.ap()[:, sl:sl + CHG, :], src[:])
    nc.compile()
    return nc


def _host_inputs(t, lut, gdt_np=np.float32):
    lut_rep = np.ascontiguousarray(lut.T[np.arange(P) % 16 % 3].astype(gdt_np))
    tf = np.ascontiguousarray(np.asarray(t, np.float32)).reshape(-1)
    # SBUF partition 16c+p slot s <- element 2048c + 16s + p of the core chunk
    tperm = (tf.reshape(N_CORES, 8, S, 16).transpose(0, 1, 3, 2)
             .reshape(N_CORES, P, S))
    return tperm, lut_rep


def _host_output(raw):
    """Per-core device output [F, 8, EPC] -> (NPC, 3)."""
    return np.ascontiguousarray(
        raw.transpose(1, 2, 0).astype(np.float32)).reshape(NPC, F)


def _host_inputs_pe(t, lut):
    bf = np.dtype("bfloat16")
    qq, rr = np.meshgrid(np.arange(K_PE), np.arange(3 * R_PE), indexing="ij")
    lrp = lut[R_PE * qq + rr // 3, rr % 3].astype(bf)
    io64 = np.arange(K_PE, dtype=np.float32).reshape(K_PE, 1)
    tf = np.ascontiguousarray(np.asarray(t, np.float32)).reshape(-1)
    tperm = (tf.reshape(N_CORES, 8, S, 16).transpose(0, 1, 3, 2)
             .reshape(N_CORES, P, S))
    return tperm, lrp, io64


def _host_output_pe(raw):
    """[P, S, 3] bf16, res[p, s] = element 2048*(p//16) + 16*s + (p%16)."""
    arr = raw.astype(np.float32).reshape(P * S, F)
    pp = np.arange(P)[:, None]
    ss = np.arange(S)[None, :]
    e = ((pp // 16) * 2048 + ss * 16 + (pp % 16)).ravel()
    out = np.empty((NPC, F), np.float32)
    out[e] = arr
    return out


def kernel(t, W1, b1, W2, b2, W3, b3):
    global LAST_RESULTS
    key = ("nc", GATHER_IMPL)
    if key not in _CACHE:
        _CACHE[key] = (_build_pe() if GATHER_IMPL.startswith("pe")
                       else _build_nc(GATHER_IMPL))
    nc = _CACHE[key]

    lut = _build_lut(np.asarray(W1, np.float32), np.asarray(b1, np.float32),
                     np.asarray(W2, np.float32), np.asarray(b2, np.float32),
                     np.asarray(W3, np.float32), np.asarray(b3, np.float32))
    if GATHER_IMPL.startswith("pe"):
        tperm, lrp, io64 = _host_inputs_pe(t, lut)
        in_maps = [{"t": np.ascontiguousarray(tperm[m]),
                    "t2": np.ascontiguousarray(tperm[m].T),
                    "lrp": lrp, "io64": io64}
                   for m in range(N_CORES)]
        res = run_bass_kernel_spmd(nc, in_maps, list(range(N_CORES)),
                                   **RUN_KWARGS)
        LAST_RESULTS = res
        outs = [_host_output_pe(res.results[m]["out"])
                for m in range(N_CORES)]
        return (np.concatenate(outs, axis=0).reshape(B, T, F)
                .astype(np.float32))

    kind = _parse(GATHER_IMPL)[0]
    gdt_np = np.dtype("bfloat16") if "b" in kind else np.float32
    tperm, lut_rep = _host_inputs(t, lut, gdt_np)
    in_maps = [{"t": np.ascontiguousarray(tperm[m]), "lut": lut_rep}
               for m in range(N_CORES)]

    res = run_bass_kernel_spmd(nc, in_maps, list(range(N_CORES)), **RUN_KWARGS)
    LAST_RESULTS = res
    outs = [_host_output(res.results[m]["out"]) for m in range(N_CORES)]
    full = np.concatenate(outs, axis=0)

    # elements j >= nvalid of each 2048-element group are not gathered on
    # device (ISA dst-elem-count limit); fill them from the same LUT here
    _, nchunk, _, nvalid = _parse(GATHER_IMPL)
    if nchunk == 1 and nvalid < EPC:
        tf = np.ascontiguousarray(np.asarray(t, np.float32)).reshape(-1)
        pos = (np.arange(N).reshape(N_CORES, 8, EPC)[:, :, nvalid:]).reshape(-1)
        u = np.floor(tf[pos] * np.float32(1024.0)).astype(np.int64)
        full[pos] = lut[np.clip(u, 0, NBINS - 1)]

    return full.reshape(B, T, F).astype(np.float32)
